# revision 40
# baseline (speedup 1.0000x reference)
"""Fused attention-with-offset kernel for Trainium2, 8-core data-parallel.

Problem (per batch element b, B=8 elements -> one NeuronCore each):
    q = query @ Wq                [SQ, D]
    k = key @ Wk                  [SKV, D]
    v = value @ Wv                [SKV, D]
    scores = (q @ k^T) / sqrt(D)  [SQ, SKV]
    attn = softmax(scores) + offset @ Woff
    out = attn @ v                [SQ, D]

Kernel strategy v3:
  - offset path keeps bf16 (it carries the output magnitude ~200);
    softmax path runs fp8 + DoubleRow.
  - no DRAM staging / no XBAR: all five transposes on the PE.
  - permuted data layout: every [seq, din] activation loads with the
    "(p so) d -> p so d" mapping so each partition reads one contiguous
    32KB block (128 descriptors per tensor instead of 2048 -> SWDGE
    descgen no longer dominates the load path).  The resulting seq/din
    permutations are consistent across every matmul and are undone by
    the access pattern of the final output DMA.
  - M4 (scores^T): 3-bank [128,1536] psum tiles consumed by one big
    scalar-engine exp ACTIVATE, plus a separate 1-bank tile (computed
    first) consumed by a DVE Schraudolph fast-exp, so neither exp
    consumer throttles the PE.
  - rowsums via N=1 DoubleRow matmuls that share PE stationary weights
    with the attn@v matmuls (~30ns each); normalization is an ACT
    scaled-copy with a per-partition 1/rowsum.
"""

import os
import sys

import numpy as np

sys.path.insert(0, "/opt/trn_rl_repo")
sys.path.insert(0, "/opt/pypackages")

B, SQ, SKV, DIN, DOUT = 8, 2048, 2048, 512, 512
P = 128
SCALE = 1.0 / float(np.sqrt(DOUT))
N_CORES = 8

# Schraudolph fast-exp constants: exp(x) ~= bitcast_f32(int(A*x + Bc))
SCH_A = 12102203.1616  # 2^23 / ln 2
SCH_B = 1064866805.0   # 127*2^23 - 486411 (min max-rel-err tuning)

_CACHED = {}


def _build_bass():
    import concourse.bass as bass
    import concourse.tile as tile
    from concourse import bacc, mybir

    f32 = mybir.dt.float32
    i32 = mybir.dt.int32
    bf16 = mybir.dt.bfloat16
    f8 = mybir.dt.float8e4
    DR = mybir.MatmulPerfMode.DoubleRow
    Exp = mybir.ActivationFunctionType.Exp
    Copy = mybir.ActivationFunctionType.Copy
    ts = bass.ts

    nc = bacc.Bacc(
        "TRN2",
        target_bir_lowering=False,
        debug=False,
        enable_asserts=True,
        num_devices=N_CORES,
    )

    query = nc.dram_tensor("query", [SQ, DIN], f32, kind="ExternalInput").ap()
    key = nc.dram_tensor("key", [SKV, DIN], f32, kind="ExternalInput").ap()
    value = nc.dram_tensor("value", [SKV, DIN], f32, kind="ExternalInput").ap()
    offset = nc.dram_tensor("offset", [SQ, DIN], f32, kind="ExternalInput").ap()
    Wq = nc.dram_tensor("Wq", [DIN, DOUT], f32, kind="ExternalInput").ap()
    Wk = nc.dram_tensor("Wk", [DIN, DOUT], f32, kind="ExternalInput").ap()
    Wv = nc.dram_tensor("Wv", [DIN, DOUT], f32, kind="ExternalInput").ap()
    Woff = nc.dram_tensor("Woff", [DIN, SKV], f32, kind="ExternalInput").ap()
    out = nc.dram_tensor("out", [SQ, DOUT], f32, kind="ExternalOutput").ap()

    KI = DIN // P    # 4  din tiles
    MO = DOUT // P   # 4  dout tiles
    TQ = SQ // P     # 16 q tiles
    TK = SKV // P    # 16 kv tiles
    NQ = SQ // 512   # 4  q chunks of 512
    NG = 4           # 512-row groups per [2048, 512] activation

    # DRAM views with the partition-contiguous permutation:
    #   activations: row = 16*p + so   (32KB contiguous per partition)
    #   weights:     row = 4*p + ko    (8KB+ contiguous per partition)
    qv = query.rearrange("(p so) d -> p so d", so=16)
    kv_ = key.rearrange("(p so) d -> p so d", so=16)
    vv = value.rearrange("(p so) d -> p so d", so=16)
    ov = offset.rearrange("(p so) d -> p so d", so=16)
    outv = out.rearrange("(p so) d -> p so d", so=16)
    wqv = Wq.rearrange("(p ko) n -> p ko n", ko=4)
    wkv = Wk.rearrange("(p ko) n -> p ko n", ko=4)
    wvv = Wv.rearrange("(p ko) n -> p ko n", ko=4)
    wov = Woff.rearrange("(p ko) n -> p ko n", ko=4)

    with tile.TileContext(nc) as tc:
        import ml_dtypes as _mld

        with (
            tc.tile_pool(name="wts", bufs=1) as wts,
            tc.tile_pool(name="big", bufs=4) as big,
            tc.tile_pool(name="expp", bufs=1) as expp,
            tc.tile_pool(name="eps", bufs=2) as eps,
        ):
            # ---- constants -------------------------------------------------
            ident16_dram = nc.inline_tensor(
                np.eye(P, dtype=_mld.bfloat16), name="ident16_const"
            )
            ident16 = wts.tile([P, P], bf16, tag="ident16")
            nc.sync.dma_start(ident16[:], ident16_dram.ap())
            ones_sb = wts.tile([P, 2, 16], f8, tag="ones")
            nc.vector.memset(ones_sb[:], 1.0)
            expbias = wts.tile([P, 1], f32, tag="expbias")
            nc.vector.memset(expbias[:], -3.0)

            wq_sb = wts.tile([P, KI, DOUT], f8, tag="wq")
            wk_sb = wts.tile([P, KI, DOUT], f8, tag="wk")
            wv_sb = wts.tile([P, KI, DOUT], bf16, tag="wv")

            # psum -> sbuf copies rotate across DVE / ACT / GpSimd so no
            # single engine throttles a DMA- or PE-paced phase
            def xcopy(eng, dst, src):
                if eng is nc.scalar:
                    nc.scalar.copy(dst, src)
                else:
                    eng.tensor_copy(dst, src)

            # transpose one 512-row group of nat [128, so, 128, 4] into
            # xT[:, c, g*512:(g+1)*512]; nat free din index = 4*a + c.
            def pe_transpose_group(xT, nat, rb, g, pool, dtype, copy_engs):
                ident = ident16
                for c in range(KI):
                    pt = pool.tile([P, 512], dtype, tag="pst", name="pt")
                    for j in range(4):
                        nc.tensor.transpose(
                            pt[:, ts(j, P)], nat[:, rb + j, :, c], ident
                        )
                    xcopy(copy_engs[c % len(copy_engs)], xT[:, c, ts(g, 512)], pt[:])

            with tc.tile_pool(name="qproj", bufs=1) as qproj:
                qpT = qproj.tile([P, MO, SQ], f8, tag="qpT")
                kpT = qproj.tile([P, MO, SKV], f8, tag="kpT")

                # ---- phase S: load q/k, PE-transpose, project --------------
                with (
                    tc.tile_pool(name="qk", bufs=1) as qk,
                    tc.tile_pool(name="pstp", bufs=2, space="PSUM") as pstp,
                    tc.tile_pool(name="psA", bufs=3, space="PSUM") as psA,
                ):
                    # all activations stream on the gpsimd cast queue in
                    # arrival order (the PE pipeline follows it); the three
                    # weights load as raw f32 on the sync/scalar HWDGE
                    # queues (SWDGE cast-DMAs with small dst runs are
                    # packet-overhead-bound) and are cast by DVE/ACT.
                    qnat = [
                        qk.tile([P, 4, P, 4], bf16, tag=f"qn{g}", name=f"qnat{g}")
                        for g in range(NG)
                    ]
                    knat = [
                        qk.tile([P, 4, P, 4], bf16, tag=f"kn{g}", name=f"knat{g}")
                        for g in range(NG)
                    ]
                    qT = qk.tile([P, KI, SQ], f8, tag="qT")
                    kT = qk.tile([P, KI, SKV], f8, tag="kT")
                    wq_f = qk.tile([P, KI, DOUT], f32, tag="wqf")
                    wk_f = qk.tile([P, KI, DOUT], f32, tag="wkf")

                    nc.sync.dma_start(wq_f[:], wqv)
                    nc.scalar.dma_start(wk_f[:], wkv)
                    nc.vector.tensor_copy(wq_sb[:], wq_f[:])
                    nc.scalar.copy(wk_sb[:], wk_f[:])
                    for g in range(NG):
                        nc.gpsimd.dma_start(qnat[g][:], qv[:, ts(g, 4), :])
                    for g in range(NG):
                        nc.gpsimd.dma_start(knat[g][:], kv_[:, ts(g, 4), :])

                    def proj_group(w_sb, xT, oT, g, copy_engs):
                        for m in range(MO):
                            pt = psA.tile([P, 512], f32, tag="mm")
                            for k in range(KI // 2):
                                nc.tensor.matmul(
                                    pt[:],
                                    lhsT=w_sb[:, 2 * k : 2 * k + 2, ts(m, P)],
                                    rhs=xT[:, 2 * k : 2 * k + 2, ts(g, 512)],
                                    start=(k == 0),
                                    stop=(k == KI // 2 - 1),
                                    perf_mode=DR,
                                )
                            xcopy(copy_engs[m % len(copy_engs)],
                                  oT[:, m, ts(g, 512)], pt[:])

                    for g in range(NG):
                        pe_transpose_group(
                            qT, qnat[g], 0, g, pstp, bf16,
                            [nc.vector, nc.scalar],
                        )
                        proj_group(wq_sb, qT, qpT, g, [nc.vector, nc.scalar])
                    for g in range(NG):
                        pe_transpose_group(
                            kT, knat[g], 0, g, pstp, bf16,
                            [nc.vector, nc.scalar],
                        )
                        proj_group(wk_sb, kT, kpT, g, [nc.scalar, nc.vector])

                # ---- loads for the later phases (issue order matters) ------
                # wv loads after q/k on the gpsimd queue as plain f32 (it is
                # not needed until phase B; casting f32->bf16 on SWDGE is
                # packet-bound, so ACT does the cast instead)
                wv_f = wts.tile([P, KI, DOUT], f32, tag="wvf")
                nc.gpsimd.dma_start(wv_f[:], wvv)
                nc.scalar.copy(wv_sb[:], wv_f[:])
                vnat = big.tile([P, 4 * NG, P, 4], bf16, tag="big")
                nc.gpsimd.dma_start(vnat[:], vv)
                # woffnat free kv index = 16*a + so
                woffnat = big.tile([P, KI, P, 16], bf16, tag="big")
                nc.gpsimd.dma_start(woffnat[:], wov)
                offnat = big.tile([P, 4 * NG, P, 4], bf16, tag="big")
                nc.gpsimd.dma_start(offnat[:], ov)

                # ---- phase M4: scores^T -> exp -----------------------------
                expT = expp.tile([P, TK, SQ], f8, tag="expT")
                s1 = SCH_A * SCALE
                s2 = SCH_B - 3.0 * SCH_A
                with (
                    tc.tile_pool(name="ps4a", bufs=2, space="PSUM") as ps4a,
                    tc.tile_pool(name="ps4b", bufs=2, space="PSUM") as ps4b,
                ):
                    for mk in range(TK):
                        pta = ps4a.tile([P, 1536], f32, tag="m4a")
                        ptb = ps4b.tile([P, 512], f32, tag="m4b")
                        # chunk 3 first: DVE fast-exp input is ready early
                        for n in (3, 0, 1, 2):
                            dst = ptb[:] if n == 3 else pta[:, ts(n, 512)]
                            for k in range(MO // 2):
                                nc.tensor.matmul(
                                    dst,
                                    lhsT=kpT[:, 2 * k : 2 * k + 2, ts(mk, P)],
                                    rhs=qpT[:, 2 * k : 2 * k + 2, ts(n, 512)],
                                    start=(k == 0),
                                    stop=(k == MO // 2 - 1),
                                    perf_mode=DR,
                                )
                        xint = eps.tile([P, 512], i32, tag="xint")
                        nc.vector.tensor_scalar(
                            xint[:], ptb[:], s1, s2,
                            mybir.AluOpType.mult, mybir.AluOpType.add,
                        )
                        nc.vector.tensor_copy(
                            expT[:, mk, 1536:2048], xint[:].bitcast(f32)
                        )
                        # bias -3 keeps exp outputs well inside fp8e4 range
                        # (max score*scale ~ 5.5); it cancels exactly in the
                        # rowsum normalization.
                        nc.scalar.activation(
                            expT[:, mk, 0:1536], pta[:], Exp,
                            scale=SCALE, bias=expbias[:],
                        )

            # ---- phase B: v path + offset-path weights -------------------
            with tc.tile_pool(name="vproj", bufs=1) as vproj:
                vp = vproj.tile([P, TK, DOUT], bf16, tag="vp")
                vp8 = vproj.tile([P, TK, DOUT], f8, tag="vp8")
                w3 = vproj.tile([P, KI, DOUT], bf16, tag="w3")

                with (
                    tc.tile_pool(name="pstp2", bufs=3, space="PSUM") as pstp2,
                    tc.tile_pool(name="psB", bufs=3, space="PSUM") as psB,
                ):
                    # v^T on the PE
                    vT = big.tile([P, KI, SKV], bf16, tag="big")
                    for g in range(NG):
                        pe_transpose_group(
                            vT, vnat, 4 * g, g, pstp2, bf16,
                            [nc.vector, nc.scalar],
                        )
                    # M3: v_proj [kv, dout] in bf16 (accuracy: feeds W3')
                    for mk in range(TK):
                        pt = psB.tile([P, 512], f32, tag="mm")
                        for k in range(KI):
                            nc.tensor.matmul(
                                pt[:],
                                lhsT=vT[:, k, ts(mk, P)],
                                rhs=wv_sb[:, k, :],
                                start=(k == 0),
                                stop=(k == KI - 1),
                            )
                        xcopy([nc.vector, nc.scalar][mk % 2], vp[:, mk, :], pt[:])
                        # vp8 derives from vp SBUF->SBUF on the otherwise-idle
                        # gpsimd engine (gpsimd cannot read PSUM)
                        nc.gpsimd.tensor_copy(vp8[:, mk, :], vp[:, mk, :])

                    # Woff^T on the PE: woffT[:, kk, j*128+a] = Woff[4a+j, 16*?]
                    # woffT partition a of tile kk holds Woff^T row kv=16a+kk
                    woffT = big.tile([P, TK, DIN], bf16, tag="big")
                    for kk in range(TK):
                        pt = pstp2.tile([P, 512], bf16, tag="pst")
                        for j in range(4):
                            nc.tensor.transpose(
                                pt[:, ts(j, P)], woffnat[:, j, :, kk], ident16
                            )
                        eng = nc.vector if kk % 2 == 0 else nc.scalar
                        if eng is nc.scalar:
                            nc.scalar.copy(woffT[:, kk, :], pt[:])
                        else:
                            eng.tensor_copy(woffT[:, kk, :], pt[:])

                    # W3' = Woff @ v_proj   [din, dout], bf16
                    for m in range(KI):
                        pt = psB.tile([P, 512], f32, tag="mm")
                        for kk in range(TK):
                            nc.tensor.matmul(
                                pt[:],
                                lhsT=woffT[:, kk, ts(m, P)],
                                rhs=vp[:, kk, :],
                                start=(kk == 0),
                                stop=(kk == TK - 1),
                            )
                        nc.vector.tensor_copy(w3[:, m, :], pt[:])

                    # offset^T on the PE
                    offT = big.tile([P, KI, SQ], bf16, tag="big")
                    for g in range(NG):
                        pe_transpose_group(
                            offT, offnat, 4 * g, g, pstp2,
                            bf16, [nc.vector, nc.scalar],
                        )

                # ---- phase C: attn@v + rowsum + offset bias + epilogue ----
                with (
                    tc.tile_pool(name="psO", bufs=2, space="PSUM") as psO,
                    tc.tile_pool(name="psF", bufs=2, space="PSUM") as psF,
                    tc.tile_pool(name="psR", bufs=2, space="PSUM") as psR,
                ):
                    for mq in range(TQ):
                        po = psO.tile([P, 512], f32, tag="mm")
                        prs = psR.tile([P, 1], f32, tag="rs")
                        for kk in range(TK // 2):
                            nc.tensor.matmul(
                                po[:],
                                lhsT=expT[:, 2 * kk : 2 * kk + 2, ts(mq, P)],
                                rhs=vp8[:, 2 * kk : 2 * kk + 2, :],
                                start=(kk == 0),
                                stop=(kk == TK // 2 - 1),
                                perf_mode=DR,
                            )
                            nc.tensor.matmul(
                                prs[:],
                                lhsT=expT[:, 2 * kk : 2 * kk + 2, ts(mq, P)],
                                rhs=ones_sb[:, :, :1],
                                start=(kk == 0),
                                stop=(kk == TK // 2 - 1),
                                perf_mode=DR,
                            )
                        poff = psF.tile([P, 512], f32, tag="mm")
                        for k in range(KI):
                            nc.tensor.matmul(
                                poff[:],
                                lhsT=offT[:, k, ts(mq, P)],
                                rhs=w3[:, k, :],
                                start=(k == 0),
                                stop=(k == KI - 1),
                            )
                        rc = eps.tile([P, 1], f32, tag="rc")
                        nc.vector.reciprocal(rc[:], prs[:])
                        tmp = eps.tile([P, 512], f32, tag="tmp")
                        nc.scalar.activation(tmp[:], po[:], Copy, scale=rc[:])
                        ot = eps.tile([P, 512], f32, tag="ot")
                        nc.vector.tensor_add(ot[:], tmp[:], poff[:])
                        nc.sync.dma_start(outv[:, mq, :], ot[:])

    nc.compile()
    return nc


def _get_nc():
    if "nc" not in _CACHED:
        _CACHED["nc"] = _build_bass()
    return _CACHED["nc"]


def _in_maps(inputs):
    def f32c(x):
        return np.ascontiguousarray(np.asarray(x), dtype=np.float32)

    shared = {k: f32c(inputs[k]) for k in ("Wq", "Wk", "Wv", "Woff")}
    return [
        {
            "query": f32c(inputs["query"][c]),
            "key": f32c(inputs["key"][c]),
            "value": f32c(inputs["value"][c]),
            "offset": f32c(inputs["offset"][c]),
            **shared,
        }
        for c in range(N_CORES)
    ]


def kernel(**inputs):
    from concourse.bass_utils import run_bass_kernel_spmd

    nc = _get_nc()
    res = run_bass_kernel_spmd(nc, _in_maps(inputs), list(range(N_CORES)))
    return np.stack([res.results[c]["out"] for c in range(N_CORES)], axis=0)


def _install_ntff_shim():
    """The agent image's antenv lacks axon_hooks; recreate it so
    run_bass_kernel_spmd(trace=True) can reach the NTFF profiler."""
    import sys as _sys
    import types

    if "antenv.axon_hooks" in _sys.modules:
        return
    mod = types.ModuleType("antenv.axon_hooks")
    _state = {"hook": None}
    mod.set_axon_ntff_profile_hook = lambda h: _state.__setitem__("hook", h)
    mod.get_axon_ntff_profile_hook = lambda: _state["hook"]
    _sys.modules["antenv.axon_hooks"] = mod
    try:
        from trn_agent_boot.trn_boot import _ntff_profile_via_ctypes

        mod.set_axon_ntff_profile_hook(
            _ntff_profile_via_ctypes("/opt/axon/libaxon_pjrt.so")
        )
    except Exception as e:
        print(f"ntff shim: could not install profile hook: {e}", file=sys.stderr)


def run_traced(**inputs):
    """Like kernel(), but also returns (output, exec_time_ns) via NTFF trace."""
    _install_ntff_shim()
    from concourse.bass_utils import run_bass_kernel_spmd

    nc = _get_nc()
    res = run_bass_kernel_spmd(nc, _in_maps(inputs), list(range(N_CORES)), trace=True)
    outv = np.stack([res.results[c]["out"] for c in range(N_CORES)], axis=0)
    return outv, res
